# revision 1
# baseline (speedup 1.0000x reference)
"""Masked causal self-attention (single head) on 8 Trainium2 NeuronCores.

Problem: x[4,4096,1024], mask[4,4096] (key padding), Wq/Wk/Wv[128,1024],
bq/bk/bv[128] -> out[4,4096,128]:
    q = x@Wq.T+bq; k = x@Wk.T+bk; v = x@Wv.T+bv
    out = softmax(causal_mask(q@k.T/sqrt(128)) + key_padding) @ v

Sharding (SPMD, one program on 8 cores): core c = (batch b=c//2, parity
p=c%2). Each core computes K/V for its full batch (replicated within the
pair) and handles the interleaved query 128-row tiles {2*t+p : t in 0..15}
— interleaving balances the causal (triangular) work between the pair.

Device algorithm per core:
  - All matmuls run in float32r (single-pass fp32, 1 cycle/row at moving
    free-dim >= 256, vs 4 cycles/row for exact fp32) via AP bitcasts.
  - Projections are computed transposed ([head, seq] layout) with the
    d-contraction on partitions: K^T/V^T/Q^T = W.T-chunks @ x^T-chunks,
    accumulated in PSUM over 8 d-chunks. Biases are folded in during the
    PSUM->SBUF eviction (per-partition scalar add). The 1/sqrt(128) score
    scale is folded into Wq on the host.
  - V^T is transposed back to [seq,head] tiles with the PE (stationary
    operand of attn@V).
  - Scores are computed transposed: S^T[k,q] = (K^T-tile).T @ Q^T-chunk.
    exp() runs on the scalar engine straight out of PSUM; its per-partition
    bias argument carries the key-padding mask (-1e30 for masked keys).
    Softmax max-subtraction is skipped: scores are ~N(0,1) by construction
    (matches jax softmax mathematically; no overflow in fp32).
  - The causal mask is data-driven so the program is core-uniform: a 0/1
    tile M = (qg >= kg) (qg per-core query indices from DRAM, kg an iota)
    multiplies exp(S^T) for the ~diagonal k-tiles only (into a fresh tile,
    pt2, so every tile has a single writer engine).
  - attn@V accumulates transposed, a whole 512-query chunk at a time:
    outT[h, qchunk] += V_kt.T @ PT_kt and den[*, qchunk] += ones.T @ PT_kt
    (the ones-matmul gives the softmax denominator replicated across
    partitions, so normalization is a plain elementwise multiply).
    The output leaves the device as [H, NQ]; the host transposes.

Hardware instructions carry a single semaphore-wait slot; Bacc.compile()
legalizes multi-wait instructions (generate_event_semaphores).
"""

import sys

sys.path.insert(0, "/opt/trn_rl_repo")

import numpy as np

import concourse.bass as bass
import concourse.bacc as bacc
import concourse.tile as tile
from concourse import mybir
from concourse.masks import make_identity
from concourse import bass_utils

F32 = mybir.dt.float32
F32R = mybir.dt.float32r
B, S, D, H = 4, 4096, 1024, 128
NQ = S // 2          # queries owned per core (2048)
DC = D // 128        # 8 d-chunks
SCH = S // 512       # 8 seq chunks of 512
NKT = S // 128       # 32 key tiles
NEG = -1.0e30


def _build_program():
    nc = bacc.Bacc("TRN2", target_bir_lowering=False)

    xT_d = nc.dram_tensor("xT", [D, S], F32R, kind="ExternalInput")
    xqT_d = nc.dram_tensor("xqT", [D, NQ], F32R, kind="ExternalInput")
    wqT_d = nc.dram_tensor("wqT", [128, DC * H], F32R, kind="ExternalInput")
    wkT_d = nc.dram_tensor("wkT", [128, DC * H], F32R, kind="ExternalInput")
    wvT_d = nc.dram_tensor("wvT", [128, DC * H], F32R, kind="ExternalInput")
    bq_d = nc.dram_tensor("bq", [H, 1], F32, kind="ExternalInput")
    bk_d = nc.dram_tensor("bk", [H, 1], F32, kind="ExternalInput")
    bv_d = nc.dram_tensor("bv", [H, 1], F32, kind="ExternalInput")
    mb_d = nc.dram_tensor("maskbias", [128, NKT], F32, kind="ExternalInput")
    qg_d = nc.dram_tensor("qg", [4, 512], F32, kind="ExternalInput")
    o_d = nc.dram_tensor("o", [H, NQ], F32, kind="ExternalOutput")

    with tile.TileContext(nc) as tc:
        with (
            tc.tile_pool(name="consts", bufs=1) as consts,
            tc.tile_pool(name="big", bufs=1) as big,
            tc.tile_pool(name="vtiles", bufs=NKT) as vtiles,
            tc.tile_pool(name="ptp", bufs=6) as ptp,
            tc.tile_pool(name="pt2p", bufs=4) as pt2p,
        ):
            # ---- constants ----
            ident = consts.tile([128, 128], F32)
            make_identity(nc, ident)
            ones_f = consts.tile([128, 128], F32, tag="ones_f")
            nc.vector.memset(ones_f, 1.0)
            ones = consts.tile([128, 128], F32R)
            nc.vector.tensor_copy(ones, ones_f)
            kg = consts.tile([128, NKT], F32)
            nc.gpsimd.iota(
                kg, pattern=[[128, NKT]], base=0, channel_multiplier=1,
                allow_small_or_imprecise_dtypes=True,
            )
            mb = consts.tile([128, NKT], F32)
            qg_b = []
            for _ in range(4):
                qg_t = consts.tile([128, 512], F32, tag="qg_b")
                qg_b.append(qg_t)

            def load_small_consts():
                nc.sync.dma_start(out=mb, in_=mb_d[:, :])
                for jj in range(4):
                    row = qg_d[jj, :]
                    nc.gpsimd.dma_start(
                        out=qg_b[jj],
                        in_=bass.AP(tensor=row.tensor, offset=row.offset,
                                    ap=[[0, 128]] + list(row.ap)),
                    )
            w_sb = {}
            for name, dram in (("q", wqT_d), ("k", wkT_d), ("v", wvT_d)):
                t = consts.tile([128, DC, H], F32R, tag=f"w_{name}")
                nc.sync.dma_start(out=t, in_=dram[:, :].rearrange("p (c h) -> p c h", c=DC))
                w_sb[name] = t
            b_sb = {}
            for name, dram in (("q", bq_d), ("k", bk_d), ("v", bv_d)):
                t = consts.tile([H, 1], F32, tag=f"b_{name}")
                nc.sync.dma_start(out=t, in_=dram[:, :])
                b_sb[name] = t
            KT = big.tile([128, S], F32R, tag="KT")     # K^T [h, k]
            QT = big.tile([128, NQ], F32R, tag="QT")    # Q^T [h, q]

            # ---- interleaved projections + attention ----
            # attention chunk j only needs k-tiles 0..8j+7 (causal), i.e.
            # K/V from s-chunks 0..2j+1 and Q chunk j: project exactly the
            # two new s-chunks per block, then run the chunk's attention.
            # PE therefore has dense work while the remaining x^T streams in.
            with (
                tc.tile_pool(name="vt_sb", bufs=1) as vt_sb_pool,
                tc.tile_pool(name="xp", bufs=20) as xp,
                tc.tile_pool(name="xqp", bufs=8) as xqp,
                tc.tile_pool(name="kps", bufs=1, space="PSUM") as kps,
                tc.tile_pool(name="vps", bufs=1, space="PSUM") as vps,
                tc.tile_pool(name="qps", bufs=1, space="PSUM") as qps,
                tc.tile_pool(name="tps", bufs=1, space="PSUM") as tps,
                tc.tile_pool(name="sp", bufs=2, space="PSUM") as sp,
                tc.tile_pool(name="op", bufs=1, space="PSUM") as op,
                tc.tile_pool(name="dp", bufs=1, space="PSUM") as dp,
                tc.tile_pool(name="osb", bufs=2) as osb,
                tc.tile_pool(name="rp", bufs=2) as rp,
            ):
                VT = vt_sb_pool.tile([128, S], F32, tag="VT")
                v_t = [None] * NKT

                def project_sc(sc):
                    kpsum = kps.tile([128, 512], F32)
                    vpsum = vps.tile([128, 512], F32)
                    for dc in range(DC):
                        xt = xp.tile([128, 512], F32R, tag="xt")
                        nc.sync.dma_start(
                            out=xt,
                            in_=xT_d[dc * 128:(dc + 1) * 128, sc * 512:(sc + 1) * 512],
                        )
                        nc.tensor.matmul(kpsum, w_sb["k"][:, dc, :], xt,
                                         start=(dc == 0), stop=(dc == DC - 1))
                        nc.tensor.matmul(vpsum, w_sb["v"][:, dc, :], xt,
                                         start=(dc == 0), stop=(dc == DC - 1))
                    nc.vector.tensor_scalar_add(
                        KT[:, sc * 512:(sc + 1) * 512], kpsum, b_sb["k"])
                    nc.vector.tensor_scalar_add(
                        VT[:, sc * 512:(sc + 1) * 512], vpsum, b_sb["v"])
                    for kt in range(4 * sc, 4 * sc + 4):
                        tpsum = tps.tile([128, 128], F32)
                        nc.tensor.transpose(
                            tpsum, VT[:, kt * 128:(kt + 1) * 128], ident)
                        vt = vtiles.tile([128, H], F32R, tag="v_t")
                        nc.vector.tensor_copy(vt, tpsum)
                        v_t[kt] = vt

                def project_q(jc):
                    qpsum = qps.tile([128, 512], F32)
                    for dc in range(DC):
                        xqt = xqp.tile([128, 512], F32R, tag="xqt")
                        nc.sync.dma_start(
                            out=xqt,
                            in_=xqT_d[dc * 128:(dc + 1) * 128, jc * 512:(jc + 1) * 512],
                        )
                        nc.tensor.matmul(qpsum, w_sb["q"][:, dc, :], xqt,
                                         start=(dc == 0), stop=(dc == DC - 1))
                    nc.vector.tensor_scalar_add(
                        QT[:, jc * 512:(jc + 1) * 512], qpsum, b_sb["q"])

                for j in range(4):
                    project_sc(2 * j)
                    if j == 0:
                        load_small_consts()
                    project_q(j)
                    project_sc(2 * j + 1)

                    n_kt = 8 * j + 8
                    outp = op.tile([128, 512], F32)
                    denp = dp.tile([128, 512], F32)
                    pts = [None] * n_kt

                    def score_exp(kt, j=j, pts=pts):
                        spsum = sp.tile([128, 512], F32)
                        nc.tensor.matmul(
                            spsum, KT[:, kt * 128:(kt + 1) * 128],
                            QT[:, j * 512:(j + 1) * 512], start=True, stop=True)
                        pt = ptp.tile([128, 512], F32R, tag="pt")
                        nc.scalar.activation(
                            pt, spsum, mybir.ActivationFunctionType.Exp,
                            bias=mb[:, kt:kt + 1], scale=1.0)
                        if kt >= 8 * j:
                            pt2 = pt2p.tile([128, 512], F32R, tag="pt2")
                            nc.vector.scalar_tensor_tensor(
                                pt2, qg_b[j], kg[:, kt:kt + 1], pt,
                                mybir.AluOpType.is_ge, mybir.AluOpType.mult)
                            pts[kt] = pt2
                        else:
                            pts[kt] = pt

                    def pv(kt, j=j, pts=pts, outp=outp, denp=denp, n_kt=n_kt):
                        nc.tensor.matmul(outp, v_t[kt], pts[kt],
                                         start=(kt == 0), stop=(kt == n_kt - 1))
                        nc.tensor.matmul(denp, ones, pts[kt],
                                         start=(kt == 0), stop=(kt == n_kt - 1))

                    # software-pipelined: PE does scores(kt+1) while the
                    # scalar engine exps scores(kt); PV lags one step
                    score_exp(0)
                    for kt in range(1, n_kt):
                        score_exp(kt)
                        pv(kt - 1)
                    pv(n_kt - 1)

                    r_t = rp.tile([128, 512], F32, tag="r")
                    nc.vector.reciprocal(r_t, denp)
                    o_sb = osb.tile([128, 512], F32, tag="o")
                    nc.vector.tensor_mul(o_sb, outp, r_t)
                    nc.sync.dma_start(
                        out=o_d[:, j * 512:(j + 1) * 512], in_=o_sb)
    nc.compile()
    return nc


def check_matmul_waits(nc, limit=1):
    bad = []
    for f in nc.m.functions:
        for bb in f.blocks:
            for i in bb.instructions:
                if i.opcode == "Matmult":
                    w = i.sync_info.on_wait if i.sync_info else []
                    if len(w) > limit:
                        bad.append((i.name, [(x.ant_name, x.wait_value) for x in w]))
    return bad


_NC_CACHE = {}


def _get_program():
    if "nc" not in _NC_CACHE:
        _NC_CACHE["nc"] = _build_program()
    return _NC_CACHE["nc"]


def _make_in_maps(x, mask, Wq, bq, Wk, bk, Wv, bv):
    x = np.asarray(x, np.float32)
    mask = np.asarray(mask)
    scale = 1.0 / np.sqrt(np.float32(H))
    def pack_w(w):
        # [H,D] -> w.T [D,H] -> partition-major [128, DC*H] for a single
        # contiguous-burst DMA into the SBUF weight tile
        wT = np.asarray(w, np.float32).T.reshape(DC, 128, H)
        return np.ascontiguousarray(wT.transpose(1, 0, 2).reshape(128, DC * H))

    wqT = pack_w(np.asarray(Wq, np.float32) * scale)
    wkT = pack_w(Wk)
    wvT = pack_w(Wv)
    bq_c = (np.asarray(bq, np.float32) * scale).reshape(H, 1).copy()
    bk_c = np.asarray(bk, np.float32).reshape(H, 1).copy()
    bv_c = np.asarray(bv, np.float32).reshape(H, 1).copy()

    in_maps = []
    for c in range(8):
        b, p = c // 2, c % 2
        xT = np.ascontiguousarray(x[b].T)                      # [D, S]
        gt = 2 * np.arange(16) + p                             # owned global q-tiles
        cols = (gt[:, None] * 128 + np.arange(128)[None, :]).reshape(-1)
        xqT = np.ascontiguousarray(xT[:, cols])                # [D, NQ]
        mbias = np.where(mask[b] == 0, np.float32(NEG), np.float32(0.0))
        mbias = np.ascontiguousarray(mbias.reshape(NKT, 128).T.astype(np.float32))
        qg = cols.reshape(4, 512).astype(np.float32)
        in_maps.append({
            "xT": xT, "xqT": xqT, "wqT": wqT, "wkT": wkT, "wvT": wvT,
            "bq": bq_c, "bk": bk_c, "bv": bv_c, "maskbias": mbias,
            "qg": np.ascontiguousarray(qg),
        })
    return in_maps


def _install_ntff_hook():
    # the trimmed antenv package lacks axon_hooks; recreate it and wire the
    # ctypes NTFF profiling hook from trn_agent_boot so trace=True works
    import types
    if "antenv.axon_hooks" in sys.modules:
        return
    import antenv
    mod = types.ModuleType("antenv.axon_hooks")
    _hook = [None]
    mod.set_axon_ntff_profile_hook = lambda h: _hook.__setitem__(0, h)
    mod.get_axon_ntff_profile_hook = lambda: _hook[0]
    sys.modules["antenv.axon_hooks"] = mod
    antenv.axon_hooks = mod
    from trn_agent_boot.trn_boot import _ntff_profile_via_ctypes
    mod.set_axon_ntff_profile_hook(
        _ntff_profile_via_ctypes("/opt/axon/libaxon_pjrt.so"))


def run(inputs, trace=False, tmpdir=None):
    if trace:
        try:
            _install_ntff_hook()
        except Exception as e:
            print("ntff hook install failed:", e)
    nc = _get_program()
    in_maps = _make_in_maps(**inputs)
    res = bass_utils.run_bass_kernel_spmd(
        nc, in_maps, core_ids=list(range(8)), trace=trace, tmpdir=tmpdir)
    out = np.empty((B, S, H), np.float32)
    for c in range(8):
        b, p = c // 2, c % 2
        o = res.results[c]["o"]                                # [H, NQ]
        for lt in range(16):
            g = 2 * lt + p
            out[b, g * 128:(g + 1) * 128, :] = o[:, lt * 128:(lt + 1) * 128].T
    return out, res


def kernel(**inputs) -> np.ndarray:
    out, _ = run(inputs, trace=False)
    return out



# revision 2
# speedup vs baseline: 1.3532x; 1.3532x over previous
"""Masked causal self-attention (single head) on 8 Trainium2 NeuronCores.

Problem: x[4,4096,1024], mask[4,4096] (key padding), Wq/Wk/Wv[128,1024],
bq/bk/bv[128] -> out[4,4096,128]:
    q = x@Wq.T+bq; k = x@Wk.T+bk; v = x@Wv.T+bv
    out = softmax(causal_mask(q@k.T/sqrt(128)) + key_padding) @ v

Sharding (SPMD, one program on 8 cores): core c = (batch b=c//2, parity
p=c%2). Each core computes K/V for its full batch (replicated within the
pair) and handles the interleaved query 128-row tiles {2*t+p : t in 0..15}
- interleaving balances the causal (triangular) work between the pair.

All PE operands are bf16 (PE internally computes at FP22 for both fp32r and
bf16, so this costs only storage precision ~0.4%, well inside tolerance):
bf16 enables Fast-Weight-Load (halves the serialized LDWEIGHTS cost that
dominated the fp32r version), halves DMA bytes and doubles DVE throughput.

Host-side packing (per batch, per parity):
  - xp[128, sc(8)*dc(8)*pos(4)*128] bf16: x^T tiled so one dma_start
    delivers [128, 4dc, 512] with d-chunk on partitions and 4KB/partition
    contiguous bursts. Within each 512 s-chunk the four 128-subtiles are
    permuted by sigma_p ([0,1,2,3] for parity 0, [1,0,3,2] for parity 1) so
    the core's OWNED q-subtiles sit at fixed positions 0 and 2 - this makes
    the single SPMD program parity-independent (Q is projected from the same
    x tiles with a strided moving AP; no separate xq input). K/V/score
    k-tile order is this permuted order; the key-padding bias (mb) and the
    causal 0/1 patterns (patt) are host-built in the same order.
  - Q = Wq-scaled projection of owned columns only; 1/sqrt(128) folded into
    Wq/bq on the host.

Device schedule per core (per block j of 512 owned queries):
  project s-chunks 2j, 2j+1 (K,V) -> project Q chunk j -> attention:
  scores^T[k,q] = KT-tile.T @ QT-chunk (PSUM f32), exp on the scalar engine
  with per-partition key-padding bias, causal mask as a precomputed-pattern
  bf16 multiply on DVE for the 8 diagonal k-tiles only, then
  outT[h,q] += v_t[kt].T @ PT and den += ones.T @ PT accumulate in PSUM.
  Epilogue: reciprocal + multiply, output [H, NQ] f32, host transposes.
  V^T tiles are PE-transposed back to [s,h] for use as the PV stationary.

Startup: 24 dummy ident@ones matmuls warm the PE HAM clock-gate (cold PE
runs at 1.2 GHz for the first ~3.4us) and a dummy exp preloads the scalar
engine's activation table (~2.7us one-time) while the first DMAs land.
Output DMAs ride the (idle) GpSimd queue so they are not stuck behind the
x-load triggers on the Sync queue.
"""

import sys

sys.path.insert(0, "/opt/trn_rl_repo")

import numpy as np
import ml_dtypes

import concourse.bass as bass
import concourse.bacc as bacc
import concourse.tile as tile
from concourse import mybir
from concourse.masks import make_identity
from concourse import bass_utils

F32 = mybir.dt.float32
BF16 = mybir.dt.bfloat16
BF16_NP = ml_dtypes.bfloat16
B, S, D, H = 4, 4096, 1024, 128
NQ = S // 2          # queries owned per core (2048)
DC = D // 128        # 8 d-chunks
SCH = S // 512       # 8 seq chunks of 512
NKT = S // 128       # 32 key tiles
NEG = -1.0e30


def _build_program():
    nc = bacc.Bacc("TRN2", target_bir_lowering=False)

    xp_d = nc.dram_tensor("xp", [128, SCH * DC * 512], BF16, kind="ExternalInput")
    wq_d = nc.dram_tensor("wq", [128, DC * H], BF16, kind="ExternalInput")
    wk_d = nc.dram_tensor("wk", [128, DC * H], BF16, kind="ExternalInput")
    wv_d = nc.dram_tensor("wv", [128, DC * H], BF16, kind="ExternalInput")
    bq_d = nc.dram_tensor("bq", [H, 1], F32, kind="ExternalInput")
    bk_d = nc.dram_tensor("bk", [H, 1], F32, kind="ExternalInput")
    bv_d = nc.dram_tensor("bv", [H, 1], F32, kind="ExternalInput")
    mb_d = nc.dram_tensor("maskbias", [128, NKT], F32, kind="ExternalInput")
    pt_d = nc.dram_tensor("patt", [128, 8 * 512], BF16, kind="ExternalInput")
    o_d = nc.dram_tensor("o", [H, NQ], F32, kind="ExternalOutput")

    with tile.TileContext(nc) as tc:
        with (
            tc.tile_pool(name="consts", bufs=1) as consts,
            tc.tile_pool(name="big", bufs=1) as big,
            tc.tile_pool(name="vtiles", bufs=NKT) as vtiles,
            tc.tile_pool(name="ptp", bufs=6) as ptp,
            tc.tile_pool(name="pt2p", bufs=4) as pt2p,
        ):
            # ---- engine-generated constants (no DMA dependence) ----
            ident = consts.tile([128, 128], BF16)
            make_identity(nc, ident)
            ones = consts.tile([128, 128], BF16, tag="ones")
            nc.vector.memset(ones, 1.0)
            act_warm = consts.tile([128, 1], F32, tag="act_warm")
            nc.scalar.activation(
                act_warm, ident[:, 0:1], mybir.ActivationFunctionType.Exp)

            # ---- input DMAs: weights/consts on the gpsimd queue ----
            w_sb = {}
            for name, dram in (("k", wk_d), ("v", wv_d), ("q", wq_d)):
                t = consts.tile([128, DC, H], BF16, tag=f"w_{name}")
                nc.gpsimd.dma_start(
                    out=t, in_=dram[:, :].rearrange("p (c h) -> p c h", c=DC))
                w_sb[name] = t
            b_sb = {}
            for name, dram in (("k", bk_d), ("v", bv_d), ("q", bq_d)):
                t = consts.tile([H, 1], F32, tag=f"b_{name}")
                nc.gpsimd.dma_start(out=t, in_=dram[:, :])
                b_sb[name] = t
            mb = consts.tile([128, NKT], F32, tag="mb")
            patt = consts.tile([128, 8, 512], BF16, tag="patt")

            def load_small_consts():
                nc.gpsimd.dma_start(out=mb, in_=mb_d[:, :])
                nc.gpsimd.dma_start(
                    out=patt, in_=pt_d[:, :].rearrange("p (r s) -> p r s", r=8))

            KT = big.tile([128, S], BF16, tag="KT")     # K^T [h, k]
            QT = big.tile([128, NQ], BF16, tag="QT")    # Q^T [h, q]

            # ---- PE/HAM warmup: ~24 back-to-back dummy matmuls ----
            with tc.tile_pool(name="warm", bufs=2, space="PSUM") as warm:
                for _ in range(24):
                    wp = warm.tile([128, 128], F32)
                    nc.tensor.matmul(wp, ident, ones, start=True, stop=True)

            # ---- interleaved projections + attention ----
            with (
                tc.tile_pool(name="vt_sb", bufs=1) as vt_sb_pool,
                tc.tile_pool(name="xp", bufs=12) as xpp,
                tc.tile_pool(name="kps", bufs=1, space="PSUM") as kps,
                tc.tile_pool(name="vps", bufs=1, space="PSUM") as vps,
                tc.tile_pool(name="qps", bufs=1, space="PSUM") as qps,
                tc.tile_pool(name="tps", bufs=1, space="PSUM") as tps,
                tc.tile_pool(name="sp", bufs=2, space="PSUM") as sp,
                tc.tile_pool(name="op", bufs=1, space="PSUM") as op,
                tc.tile_pool(name="dp", bufs=1, space="PSUM") as dp,
                tc.tile_pool(name="osb", bufs=2) as osb,
                tc.tile_pool(name="rp", bufs=2) as rp,
            ):
                VT = vt_sb_pool.tile([128, S], BF16, tag="VT")
                v_t = [None] * NKT
                x_sb = {}  # (sc, half) -> SBUF tile [128, 4, 512]

                def load_x(sc, dh):
                    xt = xpp.tile([128, 4, 512], BF16, tag="xt")
                    lo = sc * (DC * 512) + dh * (4 * 512)
                    nc.sync.dma_start(
                        out=xt,
                        in_=xp_d[:, lo:lo + 4 * 512].rearrange(
                            "p (c s) -> p c s", c=4))
                    x_sb[(sc, dh)] = xt

                def project_sc(sc):
                    kpsum = kps.tile([128, 512], F32)
                    vpsum = vps.tile([128, 512], F32)
                    for dh in range(2):
                        load_x(sc, dh)
                        xt = x_sb[(sc, dh)]
                        for dcl in range(4):
                            dc = dh * 4 + dcl
                            nc.tensor.matmul(
                                kpsum, w_sb["k"][:, dc, :], xt[:, dcl, :],
                                start=(dc == 0), stop=(dc == DC - 1))
                            nc.tensor.matmul(
                                vpsum, w_sb["v"][:, dc, :], xt[:, dcl, :],
                                start=(dc == 0), stop=(dc == DC - 1))
                    nc.vector.tensor_scalar_add(
                        KT[:, sc * 512:(sc + 1) * 512], kpsum, b_sb["k"])
                    nc.vector.tensor_scalar_add(
                        VT[:, sc * 512:(sc + 1) * 512], vpsum, b_sb["v"])
                    for kt in range(4 * sc, 4 * sc + 4):
                        tpsum = tps.tile([128, 128], BF16)
                        nc.tensor.transpose(
                            tpsum, VT[:, kt * 128:(kt + 1) * 128], ident)
                        vt = vtiles.tile([128, H], BF16, tag="v_t")
                        nc.vector.tensor_copy(vt, tpsum)
                        v_t[kt] = vt

                def owned_cols(xt, dcl):
                    # moving AP over the core's two owned 128-subtiles
                    # (positions 0 and 2 of the chunk): [128, 2, 128]
                    row = xt[:, dcl, :]
                    return bass.AP(
                        tensor=row.tensor, offset=row.offset,
                        ap=[list(row.ap[0]), [256, 2], [1, 128]])

                def project_q(j):
                    qpsum = qps.tile([128, 512], F32)
                    for e in range(2):
                        for dh in range(2):
                            xt = x_sb[(2 * j + e, dh)]
                            for dcl in range(4):
                                dc = dh * 4 + dcl
                                nc.tensor.matmul(
                                    qpsum[:, e * 256:(e + 1) * 256],
                                    w_sb["q"][:, dc, :], owned_cols(xt, dcl),
                                    start=(dc == 0), stop=(dc == DC - 1))
                    nc.vector.tensor_scalar_add(
                        QT[:, j * 512:(j + 1) * 512], qpsum, b_sb["q"])

                for j in range(4):
                    project_sc(2 * j)
                    if j == 0:
                        load_small_consts()
                    project_sc(2 * j + 1)
                    project_q(j)

                    n_kt = 8 * j + 8
                    outp = op.tile([128, 512], F32)
                    denp = dp.tile([128, 512], F32)
                    pts = [None] * n_kt

                    def score_exp(kt, j=j, pts=pts):
                        spsum = sp.tile([128, 512], F32)
                        nc.tensor.matmul(
                            spsum, KT[:, kt * 128:(kt + 1) * 128],
                            QT[:, j * 512:(j + 1) * 512], start=True, stop=True)
                        pt = ptp.tile([128, 512], BF16, tag="pt")
                        nc.scalar.activation(
                            pt, spsum, mybir.ActivationFunctionType.Exp,
                            bias=mb[:, kt:kt + 1], scale=1.0)
                        if kt >= 8 * j:
                            pt2 = pt2p.tile([128, 512], BF16, tag="pt2")
                            nc.vector.tensor_mul(pt2, pt, patt[:, kt - 8 * j, :])
                            pts[kt] = pt2
                        else:
                            pts[kt] = pt

                    def pv(kt, j=j, pts=pts, outp=outp, denp=denp, n_kt=n_kt):
                        nc.tensor.matmul(outp, v_t[kt], pts[kt],
                                         start=(kt == 0), stop=(kt == n_kt - 1))
                        nc.tensor.matmul(denp, ones, pts[kt],
                                         start=(kt == 0), stop=(kt == n_kt - 1))

                    # software-pipelined: PE does scores(kt+1) while the
                    # scalar engine exps scores(kt); PV lags one step
                    score_exp(0)
                    for kt in range(1, n_kt):
                        score_exp(kt)
                        pv(kt - 1)
                    pv(n_kt - 1)

                    r_t = rp.tile([128, 512], F32, tag="r")
                    nc.vector.reciprocal(r_t, denp)
                    o_sb = osb.tile([128, 512], F32, tag="o")
                    nc.vector.tensor_mul(o_sb, outp, r_t)
                    nc.gpsimd.dma_start(
                        out=o_d[:, j * 512:(j + 1) * 512], in_=o_sb)
    nc.compile()
    return nc


_NC_CACHE = {}


def _get_program():
    if "nc" not in _NC_CACHE:
        _NC_CACHE["nc"] = _build_program()
    return _NC_CACHE["nc"]


def _sigma(p):
    # within-chunk subtile permutation: owned subtiles at positions 0, 2
    return [0, 1, 2, 3] if p == 0 else [1, 0, 3, 2]


def _make_in_maps(x, mask, Wq, bq, Wk, bk, Wv, bv):
    x = np.asarray(x, np.float32)
    mask = np.asarray(mask)
    scale = 1.0 / np.sqrt(np.float32(H))

    def pack_w(w):
        # [H,D] -> w.T [D,H] -> partition-major [128, DC*H]
        wT = np.asarray(w, np.float32).T.reshape(DC, 128, H)
        return np.ascontiguousarray(
            wT.transpose(1, 0, 2).reshape(128, DC * H).astype(BF16_NP))

    wq = pack_w(np.asarray(Wq, np.float32) * scale)
    wk = pack_w(Wk)
    wv = pack_w(Wv)
    bq_c = (np.asarray(bq, np.float32) * scale).reshape(H, 1).copy()
    bk_c = np.asarray(bk, np.float32).reshape(H, 1).copy()
    bv_c = np.asarray(bv, np.float32).reshape(H, 1).copy()

    # per (batch, parity) packed x^T: [dp, sc, dc, pos, sf]
    xp_cache = {}

    def pack_x(b, p):
        if (b, p) not in xp_cache:
            xb = x[b].astype(BF16_NP)               # [s, d]
            xr = xb.reshape(SCH, 4, 128, DC, 128)   # [sc, t, sf, dc, dp]
            xr = xr[:, _sigma(p), :, :, :]          # [sc, pos, sf, dc, dp]
            xp_cache[(b, p)] = np.ascontiguousarray(
                xr.transpose(4, 0, 3, 1, 2).reshape(128, SCH * DC * 512))
        return xp_cache[(b, p)]

    patt_cache = {}

    def pack_patt(p):
        # patt[kp, r, i*128+qf]: causal 0/1 for diagonal k-tile offset r
        if p not in patt_cache:
            sig = _sigma(p)
            kp = np.arange(128)[:, None, None, None]
            r = np.arange(8)[None, :, None, None]
            i = np.arange(4)[None, None, :, None]
            qf = np.arange(128)[None, None, None, :]
            koff = 4 * (r // 4) + np.array(sig)[r % 4]
            qoff = 2 * i + p
            m = (qoff > koff) | ((qoff == koff) & (qf >= kp))
            patt_cache[p] = np.ascontiguousarray(
                m.astype(BF16_NP).reshape(128, 8 * 512))
        return patt_cache[p]

    in_maps = []
    for c in range(8):
        b, p = c // 2, c % 2
        sig = np.array(_sigma(p))
        kt = np.arange(NKT)
        g_kt = 4 * (kt // 4) + sig[kt % 4]          # global tile of k-tile kt
        key_idx = g_kt[None, :] * 128 + np.arange(128)[:, None]
        mbias = np.where(np.asarray(mask[b])[key_idx] == 0,
                         np.float32(NEG), np.float32(0.0)).astype(np.float32)
        in_maps.append({
            "xp": pack_x(b, p), "wq": wq, "wk": wk, "wv": wv,
            "bq": bq_c, "bk": bk_c, "bv": bv_c,
            "maskbias": np.ascontiguousarray(mbias),
            "patt": pack_patt(p),
        })
    return in_maps


def _install_ntff_hook():
    # the trimmed antenv package lacks axon_hooks; recreate it and wire the
    # ctypes NTFF profiling hook from trn_agent_boot so trace=True works
    import types
    if "antenv.axon_hooks" in sys.modules:
        return
    import antenv
    mod = types.ModuleType("antenv.axon_hooks")
    _hook = [None]
    mod.set_axon_ntff_profile_hook = lambda h: _hook.__setitem__(0, h)
    mod.get_axon_ntff_profile_hook = lambda: _hook[0]
    sys.modules["antenv.axon_hooks"] = mod
    antenv.axon_hooks = mod
    from trn_agent_boot.trn_boot import _ntff_profile_via_ctypes
    mod.set_axon_ntff_profile_hook(
        _ntff_profile_via_ctypes("/opt/axon/libaxon_pjrt.so"))


def run(inputs, trace=False, tmpdir=None):
    if trace:
        try:
            _install_ntff_hook()
        except Exception as e:
            print("ntff hook install failed:", e)
    nc = _get_program()
    in_maps = _make_in_maps(**inputs)
    res = bass_utils.run_bass_kernel_spmd(
        nc, in_maps, core_ids=list(range(8)), trace=trace, tmpdir=tmpdir)
    out = np.empty((B, S, H), np.float32)
    for c in range(8):
        b, p = c // 2, c % 2
        o = res.results[c]["o"]                                # [H, NQ]
        for lt in range(16):
            g = 2 * lt + p
            out[b, g * 128:(g + 1) * 128, :] = o[:, lt * 128:(lt + 1) * 128].T
    return out, res


def kernel(**inputs) -> np.ndarray:
    out, _ = run(inputs, trace=False)
    return out
